# revision 67
# baseline (speedup 1.0000x reference)
"""Trainium2 Bass kernel for a single transformer encoder layer.

Problem: B=4, S=2048, D=512, H=8 (dk=64), DFF=2048, f32 I/O.
Sharding: 8 cores = (batch b, token-half). Each core computes the full
layer for its 1024 tokens; K/V are computed for the whole 2048-token
batch on both cores of a pair (duplicated, zero communication). The
host permutes each core's context so its OWN tokens come first
(attention is order-invariant over context), letting the shared SPMD
program slice Q inputs at a fixed offset -- no xTo load.

Layout strategy (per core):
  - activations enter feature-major (xT, host-pretransposed, bf16)
  - K^T, Q^T feature-major with head-pairs stacked on 128 partitions
  - V token-major fp8e4, stored per head with an appended ones column
    [V_h|1], t2-tiles paired on a j axis for fp8 DoubleRow matmuls
  - scores computed transposed sT[t2, t1] (2-head row-packed matmuls
    into one 2-bank PSUM tile); softmax exp evicts PSUM->fp8 split
    across ACT (LUT exp) and DVE (Schraudolph bit-trick: round-to-int8
    of A8*s+B8 whose bits are fp8e4 exp(s), ~3% error, damped ~40x by
    the residual stream)
  - ctx^T via V-stationary fp8 DoubleRow matmuls (2 MACs/cell/cycle,
    halved PE streaming); the ones column makes row 64 of the ctx
    accumulator the softmax denominator Z (no separate Z matmuls)
  - DR ctx matmuls run 2 slots behind their exps (software pipeline),
    so the PE FIFO never blocks on an in-flight exp
  - 1/Z per pair: ln+exp(-x) on ACT (same table set as the softmax
    EXP), partition-broadcast via a tiny K=2 matmul; the PE-side bits
    (bch matmul + normalize) are deferred a few slots so the PE FIFO
    never waits on the Z chain
  - odd heads' normalized ctx^T moved to partitions 64-127 by SBUF DMA
  - Wo -> token-major attn_out, residual+LN1 (bn_stats; rstd=exp(-.5 ln v))
  - PE-transpose x1 -> FFN1 (bias+relu fused on DVE/ACT) -> FFN2
    (+b2 via a K=1 broadcast matmul into the same PSUM accumulation)
  - residual+LN2 -> out (token-major f32); block-1's post+FFN tail is
    interleaved (split ffn1 halves, LN chains alternating DVE/GpSimd,
    transposes emitted late) so the PE stays fed
  - bo and bv are folded into the host-side residual (softmax weights
    sum to 1, so bv rides through attention additively as bv@Wo)
  - ScalarE runs ONLY Exp/Ln (one activation-table set, no reloads);
    all other evictions/elementwise run on DVE or GpSimd.
"""

from contextlib import ExitStack

import numpy as np
import ml_dtypes

import concourse.bass as bass
import concourse.tile as tile
from concourse import mybir, bacc
from concourse.bass_utils import run_bass_kernel_spmd
from concourse.masks import make_identity

F32 = mybir.dt.float32
BF16 = mybir.dt.bfloat16
FP8 = mybir.dt.float8e4
I8 = mybir.dt.int8
AF = mybir.ActivationFunctionType
OP = mybir.AluOpType
PM = mybir.MatmulPerfMode

B, S, D = 4, 2048, 512
H, DK, DFF = 8, 64, 2048
EPS = 1e-5
P = 128
T1 = 1024          # own tokens per core
NCORES = 8

KD = D // P        # 4   d-tiles
NT2 = S // P       # 16  t2 tiles (context tokens)
NTP = NT2 // 2     # 8   t2 tile-pairs (fp8 DoubleRow j-planes)
NT1 = T1 // P      # 8   t1 tiles (own tokens)
NPAIR = H // 2     # 4   head pairs
NDFF = DFF // P    # 16  dff tiles
NB1 = T1 // 512    # 2   own-token 512-blocks
NBS = S // 512     # 4   context 512-blocks
DV1 = DK + 1       # 65  V columns incl the ones column
DRM = 72           # DR stationary slice width (2*72 active cols, 16-aligned)
DVP = 80           # V tile padded so the DoubleRow j-stride is 16B-aligned

# Schraudolph bit-trick exp on DVE: fp8e4 bits of exp(s) ~ round(A8*s + B8)
A8 = 8.0 / np.log(2.0)
B8 = 55.6
# which t2 tiles' softmax exps run on DVE (rest on ACT), per t1 block:
# block 0 has an idle-ish DVE (no injected post/FFN work) so it takes more
EXP_DVE = (frozenset({1, 3, 5, 7, 9, 11, 13, 15}),
           frozenset({1, 4, 6, 9, 11, 14}))


def emit(ctx: ExitStack, tc, io):
    nc = tc.nc

    xT, xo = io["xT"], io["xo"]
    wq, wk, wv, wo, w1, w2 = io["wq"], io["wk"], io["wv"], io["wo"], io["w1"], io["w2"]
    out = io["out"]

    const = ctx.enter_context(tc.tile_pool(name="const", bufs=1))
    persist = ctx.enter_context(tc.tile_pool(name="persist", bufs=1))
    exp_pool = ctx.enter_context(tc.tile_pool(name="exp", bufs=4))
    cxu_pool = ctx.enter_context(tc.tile_pool(name="cxu", bufs=6))
    work = ctx.enter_context(tc.tile_pool(name="work", bufs=3))
    stat = ctx.enter_context(tc.tile_pool(name="stat", bufs=4))
    norm = ctx.enter_context(tc.tile_pool(name="norm", bufs=1))
    xo_pool = ctx.enter_context(tc.tile_pool(name="xo", bufs=2))
    out_pool = ctx.enter_context(tc.tile_pool(name="out", bufs=2))

    mm_ps = ctx.enter_context(tc.tile_pool(name="mm_ps", bufs=2, space="PSUM"))
    sc_ps = ctx.enter_context(tc.tile_pool(name="sc_ps", bufs=2, space="PSUM"))
    ctx_ps = ctx.enter_context(tc.tile_pool(name="ctx_ps", bufs=1, space="PSUM"))

    # ---- persistent SBUF arrays ----
    # All weight/activation dram tensors are host-prearranged into the exact
    # SBUF layout (partition-major) so DMAs are long contiguous rows; each
    # array is split into per-k chunks so the transfers spread across many
    # DMA queues (~22 GB/s each). Issue order favors Q-projection inputs so
    # the PE starts early.
    wq_sb = persist.tile([P, KD, D], BF16, tag="wq")
    nc.sync.dma_start(wq_sb[:], wq[:, :, :])
    wk_sb = persist.tile([P, KD, D], BF16, tag="wk")
    nc.sync.dma_start(wk_sb[:], wk[:, :, :])
    xT_sb = persist.tile([P, KD, S], BF16, tag="xT")
    for half in range(2):
        hs = slice(half * T1, (half + 1) * T1)
        for k in range(KD):
            nc.sync.dma_start(xT_sb[:, k, hs], xT[:, k, hs])
    wv_sb = persist.tile([P, KD, D], BF16, tag="wv")
    nc.sync.dma_start(wv_sb[:], wv[:, :, :])
    wo_sb = persist.tile([P, KD, D], BF16, tag="wo")
    nc.sync.dma_start(wo_sb[:], wo[:, :, :])

    kt_sb = persist.tile([P, NPAIR, S], BF16, tag="kt")
    qt_sb = persist.tile([P, NPAIR, T1], BF16, tag="qt")
    # V (fp8) with per-head ones column, t2 tiles paired on the j axis for
    # DoubleRow: [t2 128, t2pair, j, head, 80]; cols 64 are ones, 65-79 pad.
    ve_sb = persist.tile([P, NTP, 2, H, DVP], FP8, tag="ve")
    nc.vector.memset(ve_sb[:, :, :, :, DK:DK + 1], 1.0)
    ctxT_sb = persist.tile([P, NPAIR, T1], BF16, tag="ctxT")
    x1_sb = persist.tile([P, NT1, D], BF16, tag="x1")
    x1T_sb = persist.tile([P, KD, T1], BF16, tag="x1T")
    h1T_sb = persist.tile([P, NDFF, T1], BF16, tag="h1T")

    # ---- constants ----
    # per-partition bias tiles (feature-major evictions) -- first so the Q/K
    # evictions are never gated on the gpsimd DMA queue
    bqt = const.tile([P, KD], F32)
    nc.gpsimd.dma_start(bqt[:], io["bq"][:].rearrange("(m p) -> p m", p=P))
    bkt = const.tile([P, KD], F32)
    nc.gpsimd.dma_start(bkt[:], io["bk"][:].rearrange("(m p) -> p m", p=P))
    b1t = const.tile([P, NDFF], F32)
    nc.gpsimd.dma_start(b1t[:], io["b1"][:].rearrange("(m p) -> p m", p=P))

    ident_sb = const.tile([P, P], BF16)
    make_identity(nc, ident_sb[:])
    # HAM warm-up: the PE would otherwise idle ~10us waiting for the first
    # weight DMAs and start the projections cold (K=4/8, 1.2GHz). ident_sb
    # needs no DMA, so a chain of back-to-back identity matmuls spans the
    # wait and flips the clock gate to 8/8 before real work arrives.
    warm_ps = mm_ps.tile([P, P], F32, tag="mm")
    for _ in range(100):
        nc.tensor.matmul(warm_ps[:], ident_sb[:], ident_sb[:],
                         start=True, stop=True)
    ones1_sb = const.tile([1, P], BF16)
    nc.vector.memset(ones1_sb[:], 1.0)
    b2r_sb = const.tile([1, D], BF16)
    nc.gpsimd.dma_start(b2r_sb[:], io["b2"][:].unsqueeze(0))
    eps_sb = const.tile([P, 1], F32)
    nc.vector.memset(eps_sb[:], EPS)
    # per-pair 1/Z partition-broadcast indicator: kron(eye(2), ones(64))
    ind2_sb = const.tile([2, P], BF16)
    nc.gpsimd.dma_start(ind2_sb[:], io["ind2"][:, :])

    # free-axis broadcast tiles (token-major ops)
    def bc_tile(name):
        t = const.tile([P, D], BF16, tag=f"bc_{name}")
        a = io[name][:]
        bcast = bass.AP(tensor=a.tensor, offset=a.offset, ap=[[0, P]] + list(a.ap))
        nc.gpsimd.dma_start(t[:], bcast)
        return t

    g1b = bc_tile("g1")
    be1b = bc_tile("be1")
    g2b = bc_tile("g2")
    be2b = bc_tile("be2")

    # ---- projections ----
    # Q^T (feature-major, head-pairs stacked): [dk-pair 128, t1];
    # only m=0 (pair 0) upfront, m=1..3 injected into block-0 attention
    def qproj(m, nb):
        ps = mm_ps.tile([P, 512], F32, tag="mm")
        for k in range(KD):
            nc.tensor.matmul(
                ps[:],
                wq_sb[:, k, m * P:(m + 1) * P],
                xT_sb[:, k, nb * 512:(nb + 1) * 512],
                start=(k == 0),
                stop=(k == KD - 1),
            )
        nc.scalar.activation(
            qt_sb[:, m, nb * 512:(nb + 1) * 512], ps[:], AF.Identity,
            bias=bqt[:, m:m + 1],
        )

    for m in range(KD):
        for nb in range(NB1):
            qproj(m, nb)
    # K^T -- m=0 (pair 0) upfront; m=1..3 injected into block-0 attention
    def kproj(m, nb):
        ps = mm_ps.tile([P, 512], F32, tag="mm")
        for k in range(KD):
            nc.tensor.matmul(
                ps[:],
                wk_sb[:, k, m * P:(m + 1) * P],
                xT_sb[:, k, nb * 512:(nb + 1) * 512],
                start=(k == 0),
                stop=(k == KD - 1),
            )
        nc.scalar.activation(
            kt_sb[:, m, nb * 512:(nb + 1) * 512], ps[:], AF.Identity,
            bias=bkt[:, m:m + 1],
        )

    for nb in range(NBS):
        kproj(0, nb)
    # V (token-major fp8, per-head strided [h, dv] groups in j-planes);
    # tiles 0..1 upfront, the rest injected into block-0 pair 0 so the
    # projection matmuls fill the exp-bound attention window.
    def vproj(i):
        ps = mm_ps.tile([P, 512], F32, tag="mm")
        for k in range(KD):
            nc.tensor.matmul(
                ps[:],
                xT_sb[:, k, i * P:(i + 1) * P],
                wv_sb[:, k, :],
                start=(k == 0),
                stop=(k == KD - 1),
            )
        # bv is folded into the residual host-side (softmax weights sum to 1,
        # so ctx = ctx_nobias + bv and bv@Wo+bo joins xo); plain fp8 evict.
        nc.scalar.activation(
            ve_sb[:, i // 2, i % 2, :, 0:DK],
            ps[:].rearrange("p (h d) -> p h d", h=H),
            AF.Identity,
        )

    vproj(0)
    vproj(1)

    # W1 shares the xT slot (xT dead once the injected projections finish);
    # the prefetch DMAs go on the gpsimd queue (their wait on the xT slot
    # must not block sync's small Z/stg/out DMAs) and are emitted at block-0
    # pair 3 via prefetch_w().
    w1_sb = persist.tile([P, KD, DFF], BF16, tag="xT")
    w2_sb = persist.tile([P, NDFF, D], BF16, tag="w2")

    def prefetch_w():
        nc.gpsimd.dma_start(w1_sb[:], w1[:, :, :])
        nc.gpsimd.dma_start(w2_sb[:], w2[:, :, :])

    def layer_norm(r, gb, beb, dest, eng=None):
        """dest = LN(r)*g + be; r is f32 SBUF [128, D]."""
        eng = eng or nc.gpsimd
        st = stat.tile([P, 6], F32, tag="st")
        nc.vector.bn_stats(st[:], r[:])
        mv = stat.tile([P, 2], F32, tag="mv")
        nc.vector.bn_aggr(mv[:], st[:])
        lnv = stat.tile([P, 1], F32, tag="lnv")
        nc.scalar.activation(lnv[:], mv[:, 1:2], AF.Ln, bias=eps_sb[:, 0:1])
        rstd = stat.tile([P, 1], F32, tag="rstd")
        nc.scalar.activation(rstd[:], lnv[:], AF.Exp, scale=-0.5)
        xc = work.tile([P, D], F32, tag="xc")
        nc.vector.tensor_scalar(
            xc[:], r[:], mv[:, 0:1], rstd[:], op0=OP.subtract, op1=OP.mult
        )
        xg = work.tile([P, D], F32, tag="xg")
        eng.tensor_tensor(xg[:], xc[:], gb[:], OP.mult)
        eng.tensor_tensor(dest, xg[:], beb[:], OP.add)

    post_stats = {}
    # prefetch all residual tiles early (DMA queues are idle mid-kernel)
    xo_tiles = {}
    for t1t in range(NT1):
        xo_t = xo_pool.tile([P, D], F32)
        nc.gpsimd.dma_start(xo_t[:], xo[t1t * P:(t1t + 1) * P, :])
        xo_tiles[t1t] = xo_t

    def post_attn1(t1t, eng=None):
        """Wo + residual (staged bf16 in the x1 slot) + bn stats."""
        eng = eng or nc.gpsimd
        ao = mm_ps.tile([P, 512], F32, tag="mm")
        for k in range(NPAIR):
            nc.tensor.matmul(
                ao[:],
                ctxT_sb[:, k, t1t * P:(t1t + 1) * P],
                wo_sb[:, k, :],
                start=(k == 0),
                stop=(k == NPAIR - 1),
            )
        xo_t = xo_tiles[t1t]
        rslot = x1_sb[:, t1t, :]
        nc.vector.tensor_tensor(rslot, ao[:], xo_t[:], OP.add)
        st = stat.tile([P, 6], F32, tag="st")
        nc.vector.bn_stats(st[:], rslot)
        mv = stat.tile([P, 2], F32, tag="mv")
        nc.vector.bn_aggr(mv[:], st[:])
        post_stats[t1t] = mv

    def post2_norm(t1t, eng=None):
        """LN1 normalize+affine (in the x1 slot)."""
        eng = eng or nc.gpsimd
        mv = post_stats.pop(t1t)
        rslot = x1_sb[:, t1t, :]
        lnv = stat.tile([P, 1], F32, tag="lnv")
        nc.scalar.activation(lnv[:], mv[:, 1:2], AF.Ln, bias=eps_sb[:, 0:1])
        rstd = stat.tile([P, 1], F32, tag="rstd")
        nc.scalar.activation(rstd[:], lnv[:], AF.Exp, scale=-0.5)
        xc = work.tile([P, D], F32, tag="xc")
        nc.vector.tensor_scalar(
            xc[:], rslot, mv[:, 0:1], rstd[:], op0=OP.subtract, op1=OP.mult
        )
        xg = work.tile([P, D], F32, tag="xg")
        eng.tensor_tensor(xg[:], xc[:], g1b[:], OP.mult)
        eng.tensor_tensor(rslot, xg[:], be1b[:], OP.add)

    def post2_T(t1t, eng=None):
        """transpose(x1) -> x1T."""
        eng = eng or nc.gpsimd
        for j in range(KD):
            tp = mm_ps.tile([P, P], BF16, tag="mm")
            nc.tensor.transpose(
                tp[:], x1_sb[:, t1t, j * P:(j + 1) * P], ident_sb[:]
            )
            if eng is nc.vector:
                nc.scalar.copy(x1T_sb[:, j, t1t * P:(t1t + 1) * P], tp[:])
            else:
                nc.vector.tensor_copy(x1T_sb[:, j, t1t * P:(t1t + 1) * P], tp[:])

    def post_attn2(t1t, eng=None):
        post2_norm(t1t, eng)
        post2_T(t1t, eng)

    def ffn1_m(t1b, m, width=512, off=0):
        on_act = t1b == NB1 - 1
        lo = t1b * 512 + off
        if True:
            ps = mm_ps.tile([P, 512], F32, tag="mm")
            for k in range(KD):
                nc.tensor.matmul(
                    ps[:, 0:width],
                    w1_sb[:, k, m * P:(m + 1) * P],
                    x1T_sb[:, k, lo:lo + width],
                    start=(k == 0),
                    stop=(k == KD - 1),
                )
            # h1 = relu(ps + b1); ACT in the tail block (ACT idle there),
            # alternating DVE/ACT in block 0 to avoid a 12us DVE burst
            if on_act or m % 2 == 1:
                nc.scalar.activation(
                    h1T_sb[:, m, lo:lo + width], ps[:, 0:width], AF.Relu,
                    bias=b1t[:, m:m + 1],
                )
            else:
                nc.vector.tensor_scalar(
                    h1T_sb[:, m, lo:lo + width], ps[:, 0:width],
                    b1t[:, m:m + 1], 0.0, op0=OP.add, op1=OP.max,
                )

    def ffn1(t1b, width=512, off=0):
        for m in range(NDFF):
            ffn1_m(t1b, m, width, off)

    def ffn2(t1t, eng=None):
        eng = eng or nc.vector
        ff = mm_ps.tile([P, 512], F32, tag="mm")
        for k in range(NDFF):
            nc.tensor.matmul(
                ff[:],
                h1T_sb[:, k, t1t * P:(t1t + 1) * P],
                w2_sb[:, k, :],
                start=(k == 0),
                stop=False,
            )
        # + b2 via a K=1 broadcast matmul (frees the tail engine chains)
        nc.tensor.matmul(
            ff[:], ones1_sb[0:1, :], b2r_sb[0:1, :], start=False, stop=True,
        )
        r = work.tile([P, D], F32, tag=f"r2{t1t % 2}")
        nc.vector.tensor_tensor(r[:], ff[:], x1_sb[:, t1t, :], OP.add)
        o = out_pool.tile([P, D], F32)
        layer_norm(r, g2b, be2b, o[:], eng=eng)
        nc.sync.dma_start(out[t1t * P:(t1t + 1) * P, :], o[:])

    # ---- attention (t1-block outer so downstream work pipelines) ----
    pending_norm = []
    for t1b in range(NB1):
        t1s = slice(t1b * 512, (t1b + 1) * 512)
        # block-0 post/FFN spread across block-1's t2 slots one unit at a
        # time (burst-free engine queues); post_attn1 must come after the
        # block-0 pair-3 normalize flush (pair-0 slot 9).
        inj = {}
        if t1b == 1:
            inj[(0, 10)] = lambda: post_attn1(0)
            inj[(0, 12)] = lambda: post_attn1(1)
            inj[(0, 14)] = lambda: post_attn1(2)
            inj[(1, 0)] = lambda: post_attn1(3)
            inj[(1, 2)] = lambda: post2_norm(0)
            inj[(1, 4)] = lambda: post2_norm(1)
            inj[(1, 6)] = lambda: post2_norm(2)
            inj[(1, 8)] = lambda: post2_norm(3)
            inj[(1, 10)] = lambda: post2_T(0)
            inj[(1, 12)] = lambda: post2_T(1)
            inj[(1, 14)] = lambda: post2_T(2)
            inj[(2, 0)] = lambda: post2_T(3)
            for mm_ in range(1, 16):
                inj[(2, mm_)] = (lambda m=mm_ - 1: ffn1_m(0, m))
            inj[(3, 0)] = lambda: ffn1_m(0, 15)
            inj[(3, 2)] = lambda: ffn2(0, eng=nc.gpsimd)
            inj[(3, 9)] = lambda: ffn2(1, eng=nc.gpsimd)

        # Software pipeline: the DoubleRow ctx matmuls for DR-slot q run two
        # slots after their exps were emitted (at slot q+2), so the PE never
        # stalls on an in-flight exp. cx banks for pair p are freed by the
        # cu evictions emitted right after DR(p, tp=7) -- one slot before
        # DR(p+1, tp=0) needs them.
        cx = {}

        def emit_dr(q):
            p_, tp_ = divmod(q, NTP)
            hA_, hB_ = 2 * p_, 2 * p_ + 1
            if tp_ == 0:
                cxA = ctx_ps.tile([DV1, 512], F32, tag="cxA")
                cxB = ctx_ps.tile([DV1, 512], F32, tag="cxB")
                cx[p_] = (cxA, cxB)
            cxA, cxB = cx[p_]
            eAB = etile.pop(q)
            first, last = tp_ == 0, tp_ == NTP - 1
            nc.tensor.matmul(
                cxA[:, :], ve_sb[:, tp_, :, hA_, 0:DV1], eAB[:, 0, :, :],
                start=first, stop=last, perf_mode=PM.DoubleRow,
            )
            nc.tensor.matmul(
                cxB[:, :], ve_sb[:, tp_, :, hB_, 0:DV1], eAB[:, 1, :, :],
                start=first, stop=last, perf_mode=PM.DoubleRow,
            )
            if last:
                # evict unnormalized ctx (bf16, frees the cx banks), gather Z
                # rows and start 1/Z = exp(-ln Z) on ACT. The PE-side pieces
                # (bch broadcast matmuls + normalize muls) are DEFERRED a few
                # slots so the PE FIFO never blocks on this chain.
                zall2 = norm.tile([2, 512], F32, tag=f"z2_{p_ % 2}")
                cus = {}
                for h, cxt in ((hA_, cxA), (hB_, cxB)):
                    cu = cxu_pool.tile([64, 512], BF16, tag="cu")
                    nc.vector.tensor_copy(cu[:], cxt[0:64, :])
                    zst = norm.tile([P, 512], F32, tag=f"zst{h % 2}")
                    nc.scalar.copy(zst[64:65, :], cxt[64:65, :])
                    nc.sync.dma_start(zall2[h % 2:h % 2 + 1, :], zst[64:65, :])
                    cus[h] = cu
                lz2 = norm.tile([2, 512], F32, tag=f"lz_{p_ % 2}")
                nc.scalar.activation(lz2[:], zall2[:], AF.Ln)
                rz2 = norm.tile([2, 512], BF16, tag=f"rz_{p_ % 2}")
                nc.scalar.activation(rz2[:], lz2[:], AF.Exp, scale=-1.0)
                ts_ = t1s

                def do_norm(p=p_, hA=hA_, hB=hB_, rz=rz2, cus=cus, ts=ts_):
                    for h in (hA, hB):
                        odd = h % 2
                        bch = mm_ps.tile([64, 512], F32, tag="mm")
                        nc.tensor.matmul(
                            bch[:], ind2_sb[:, odd * DK:(odd + 1) * DK],
                            rz[:, :], start=True, stop=True,
                        )
                        if not odd:
                            nc.vector.tensor_tensor(
                                ctxT_sb[0:64, p, ts], cus[h][:], bch[:], OP.mult
                            )
                        else:
                            stg = work.tile([64, 512], BF16, tag="stg")
                            nc.vector.tensor_tensor(stg[:], cus[h][:], bch[:],
                                                    OP.mult)
                            nc.sync.dma_start(ctxT_sb[64:128, p, ts], stg[:])

                pending_norm.append(do_norm)

        etile = {}
        for pair in range(NPAIR):
            for t2 in range(NT2):
                t2s = slice(t2 * P, (t2 + 1) * P)
                tp, j = divmod(t2, 2)
                q = pair * NTP + tp
                sAB = sc_ps.tile([P, 2, 512], F32, tag="s")
                nc.tensor.matmul(
                    sAB[:, 0, :], kt_sb[0:64, pair, t2s], qt_sb[0:64, pair, t1s],
                    start=True, stop=True, tile_position=(0, 0),
                )
                nc.tensor.matmul(
                    sAB[:, 1, :], kt_sb[64:128, pair, t2s], qt_sb[64:128, pair, t1s],
                    start=True, stop=True, tile_position=(64, 0),
                    skip_group_check=True,
                )
                # softmax numerator in fp8, split across ACT (LUT exp) and
                # DVE (Schraudolph bit-trick via round-to-int8)
                if j == 0:
                    eAB = exp_pool.tile([P, 2, 2, 512], FP8, tag="e")
                    etile[q] = eAB
                if t2 % NT2 in EXP_DVE[t1b]:
                    nc.vector.tensor_scalar(
                        etile[q][:, :, j, :].bitcast(I8), sAB[:, :, :], A8, B8,
                        op0=OP.mult, op1=OP.add,
                    )
                else:
                    nc.scalar.activation(
                        etile[q][:, :, j, :], sAB[:, :, :], AF.Exp
                    )
                if j == 1 and q >= 2:
                    emit_dr(q - 2)
                    if (q - 2) % NTP == 2 and pending_norm:
                        pending_norm.pop(0)()
                if t1b == 0:
                    # fill block-0's exp-bound window with deferred projections
                    if pair == 0 and 1 <= t2 <= 7:
                        vproj(2 * t2)
                        vproj(2 * t2 + 1)
                    elif pair < 3 and 9 <= t2 <= 12:
                        kproj(pair + 1, t2 - 9)
                    elif pair == 3 and t2 == 0:
                        prefetch_w()
                fn = inj.get((pair, t2))
                if fn is not None:
                    fn()
        emit_dr(NPAIR * NTP - 2)
        emit_dr(NPAIR * NTP - 1)

    # tail: block-1 post + FFN, interleaved so LN chains overlap matmuls and
    # run on two engines (DVE / GpSimd) concurrently; ffn1 is split in half
    # so ffn2+LN2+store of tiles 4,5 overlap ffn1 of tiles 6,7
    ffn2(2, eng=nc.gpsimd)
    ffn2(3, eng=nc.gpsimd)
    while pending_norm:
        pending_norm.pop(0)()
    post_attn1(4, eng=nc.vector)
    post_attn1(5, eng=nc.vector)
    post2_norm(4, eng=nc.vector)
    post2_norm(5, eng=nc.gpsimd)
    post_attn1(6, eng=nc.vector)
    post_attn1(7, eng=nc.vector)
    post2_T(4, eng=nc.vector)
    post2_T(5, eng=nc.gpsimd)
    post2_norm(6, eng=nc.vector)
    post2_norm(7, eng=nc.gpsimd)
    ffn1(1, width=256, off=0)
    post2_T(6, eng=nc.vector)
    post2_T(7, eng=nc.gpsimd)
    ffn2(4, eng=nc.vector)
    ffn1(1, width=256, off=256)
    ffn2(5, eng=nc.gpsimd)
    ffn2(6, eng=nc.gpsimd)
    ffn2(7, eng=nc.vector)


def _patch_act_tables():
    """Force every ACT op onto the natural_log_exp_and_others table set so
    the kernel pays one ACT_TABLE_LOAD instead of thrashing between the
    per-function default sets (Exp<->Ln cost 33 loads / 42us)."""
    import functools
    import concourse.hw_specs as hw_specs

    if getattr(hw_specs, "_nle_only", False):
        return
    orig = hw_specs.get_activation_tables

    @functools.cache
    def nle_only(arch):
        tabs = orig(arch)
        return {
            k: (v if k == "natural_log_exp_and_others" else set())
            for k, v in tabs.items()
        }

    hw_specs.get_activation_tables = nle_only
    hw_specs._nle_only = True
    # bacc imported the symbol directly
    if getattr(bacc, "get_activation_tables", None) is not None:
        bacc.get_activation_tables = nle_only


def build_program():
    _patch_act_tables()
    nc = bacc.Bacc("TRN2", target_bir_lowering=False, debug=False, num_devices=NCORES)
    io = {}
    io["xT"] = nc.dram_tensor("xT", [P, KD, S], BF16, kind="ExternalInput").ap()
    io["xo"] = nc.dram_tensor("xo", [T1, D], F32, kind="ExternalInput").ap()
    for name, shape in [
        ("wq", [P, KD, D]), ("wk", [P, KD, D]), ("wv", [P, KD, D]),
        ("wo", [P, KD, D]), ("w1", [P, KD, DFF]), ("w2", [P, NDFF, D]),
    ]:
        io[name] = nc.dram_tensor(name, shape, BF16, kind="ExternalInput").ap()
    for name, n in [
        ("bq", D), ("bk", D), ("bv", D), ("bo", D), ("b1", DFF), ("b2", D),
        ("g1", D), ("be1", D), ("g2", D), ("be2", D),
    ]:
        io[name] = nc.dram_tensor(name, [n], F32, kind="ExternalInput").ap()
    io["ind2"] = nc.dram_tensor("ind2", [2, P], BF16, kind="ExternalInput").ap()
    io["out"] = nc.dram_tensor("out", [T1, D], F32, kind="ExternalOutput").ap()

    with tile.TileContext(nc) as tc:
        with ExitStack() as ctx:
            emit(ctx, tc, io)
    nc.compile()
    return nc


def make_in_maps(x, Wq, bq, Wk, bk, Wv, bv, Wo, bo, W1, b1, W2, b2,
                 g1, be1, g2, be2):
    bf = ml_dtypes.bfloat16
    f32 = np.float32
    scale = 1.0 / np.sqrt(DK)

    def lay(w):
        # [D_in, M] -> partition-major SBUF layout [P, D_in//P, M]
        w = np.asarray(w)
        kd = w.shape[0] // P
        return np.ascontiguousarray(w.reshape(kd, P, w.shape[1]).transpose(1, 0, 2))

    shared = {
        "wq": lay((np.asarray(Wq, f32) * scale).astype(bf)),
        "wk": lay(np.asarray(Wk, f32).astype(bf)),
        "wv": lay(np.asarray(Wv, f32).astype(bf)),
        "wo": lay(np.asarray(Wo, f32).astype(bf)),
        "w1": lay(np.asarray(W1, f32).astype(bf)),
        "w2": lay(np.asarray(W2, f32).astype(bf)),
        "bq": (np.asarray(bq, f32) * scale),
        "bk": np.asarray(bk, f32), "bv": np.asarray(bv, f32),
        "bo": np.asarray(bo, f32), "b1": np.asarray(b1, f32),
        "b2": np.asarray(b2, f32), "g1": np.asarray(g1, f32),
        "be1": np.asarray(be1, f32), "g2": np.asarray(g2, f32),
        "be2": np.asarray(be2, f32),
        "ind2": np.kron(np.eye(2, dtype=f32), np.ones((1, DK), f32)).astype(bf),
    }
    x = np.asarray(x, f32)
    # softmax weights sum to 1, so bv rides through attention additively:
    # attn_out = ctx_nobias @ Wo + (bv @ Wo + bo); fold into the residual.
    bo_f = (np.asarray(bo, f32) + np.asarray(bv, f32) @ np.asarray(Wo, f32))
    in_maps = []
    for c in range(NCORES):
        b, half = divmod(c, 2)
        xb = x[b]                                    # [S, D] f32
        sl = slice(half * T1, (half + 1) * T1)
        ot = slice((1 - half) * T1, (2 - half) * T1)
        # own tokens FIRST: attention is order-invariant over context, and
        # this lets the shared program read Q's inputs at a fixed offset
        xperm = np.concatenate([xb[sl], xb[ot]], axis=0)
        m = dict(shared)
        m["xT"] = lay(np.ascontiguousarray(xperm.T).astype(bf))
        m["xo"] = np.ascontiguousarray(xb[sl]) + bo_f  # residual with bo folded
        in_maps.append(m)
    return in_maps


_prog_cache = {}


def get_program():
    if "nc" not in _prog_cache:
        _prog_cache["nc"] = build_program()
    return _prog_cache["nc"]


def kernel(**inputs) -> np.ndarray:
    nc = get_program()
    in_maps = make_in_maps(**inputs)
    res = run_bass_kernel_spmd(nc, in_maps, core_ids=list(range(NCORES)))
    out = np.empty((B, S, D), np.float32)
    for c in range(NCORES):
        b, half = divmod(c, 2)
        out[b, half * T1:(half + 1) * T1] = res.results[c]["out"]
    return out


if __name__ == "__main__":
    rng = np.random.default_rng(0)
    print("building program...")
    get_program()
    print("built")



# revision 68
# speedup vs baseline: 1.0042x; 1.0042x over previous
"""Trainium2 Bass kernel for a single transformer encoder layer.

Problem: B=4, S=2048, D=512, H=8 (dk=64), DFF=2048, f32 I/O.
Sharding: 8 cores = (batch b, token-half). Each core computes the full
layer for its 1024 tokens; K/V are computed for the whole 2048-token
batch on both cores of a pair (duplicated, zero communication). The
host permutes each core's context so its OWN tokens come first
(attention is order-invariant over context), letting the shared SPMD
program slice Q inputs at a fixed offset -- no xTo load.

Layout strategy (per core):
  - activations enter feature-major (xT, host-pretransposed, bf16)
  - K^T, Q^T feature-major with head-pairs stacked on 128 partitions
  - V token-major fp8e4, stored per head with an appended ones column
    [V_h|1], t2-tiles paired on a j axis for fp8 DoubleRow matmuls
  - scores computed transposed sT[t2, t1] (2-head row-packed matmuls
    into one 2-bank PSUM tile); softmax exp evicts PSUM->fp8 split
    across ACT (LUT exp) and DVE (Schraudolph bit-trick: round-to-int8
    of A8*s+B8 whose bits are fp8e4 exp(s), ~3% error, damped ~40x by
    the residual stream)
  - ctx^T via V-stationary fp8 DoubleRow matmuls (2 MACs/cell/cycle,
    halved PE streaming); the ones column makes row 64 of the ctx
    accumulator the softmax denominator Z (no separate Z matmuls)
  - DR ctx matmuls run 2 slots behind their exps (software pipeline),
    so the PE FIFO never blocks on an in-flight exp
  - 1/Z per pair: ln+exp(-x) on ACT (same table set as the softmax
    EXP), partition-broadcast via a tiny K=2 matmul; the PE-side bits
    (bch matmul + normalize) are deferred a few slots so the PE FIFO
    never waits on the Z chain
  - odd heads' normalized ctx^T moved to partitions 64-127 by SBUF DMA
  - Wo -> token-major attn_out, residual+LN1 (bn_stats; rstd=exp(-.5 ln v))
  - PE-transpose x1 -> FFN1 (bias+relu fused on DVE/ACT) -> FFN2
    (+b2 via a K=1 broadcast matmul into the same PSUM accumulation)
  - residual+LN2 -> out (token-major f32); block-1's post+FFN tail is
    interleaved (split ffn1 halves, LN chains alternating DVE/GpSimd,
    transposes emitted late) so the PE stays fed
  - bo and bv are folded into the host-side residual (softmax weights
    sum to 1, so bv rides through attention additively as bv@Wo)
  - ScalarE runs ONLY Exp/Ln (one activation-table set, no reloads);
    all other evictions/elementwise run on DVE or GpSimd.
"""

from contextlib import ExitStack

import numpy as np
import ml_dtypes

import concourse.bass as bass
import concourse.tile as tile
from concourse import mybir, bacc
from concourse.bass_utils import run_bass_kernel_spmd
from concourse.masks import make_identity

F32 = mybir.dt.float32
BF16 = mybir.dt.bfloat16
FP8 = mybir.dt.float8e4
I8 = mybir.dt.int8
AF = mybir.ActivationFunctionType
OP = mybir.AluOpType
PM = mybir.MatmulPerfMode

B, S, D = 4, 2048, 512
H, DK, DFF = 8, 64, 2048
EPS = 1e-5
P = 128
T1 = 1024          # own tokens per core
NCORES = 8

KD = D // P        # 4   d-tiles
NT2 = S // P       # 16  t2 tiles (context tokens)
NTP = NT2 // 2     # 8   t2 tile-pairs (fp8 DoubleRow j-planes)
NT1 = T1 // P      # 8   t1 tiles (own tokens)
NPAIR = H // 2     # 4   head pairs
NDFF = DFF // P    # 16  dff tiles
NB1 = T1 // 512    # 2   own-token 512-blocks
NBS = S // 512     # 4   context 512-blocks
DV1 = DK + 1       # 65  V columns incl the ones column
DRM = 72           # DR stationary slice width (2*72 active cols, 16-aligned)
DVP = 80           # V tile padded so the DoubleRow j-stride is 16B-aligned

# Schraudolph bit-trick exp on DVE: fp8e4 bits of exp(s) ~ round(A8*s + B8)
A8 = 8.0 / np.log(2.0)
B8 = 55.6
# which t2 tiles' softmax exps run on DVE (rest on ACT), per t1 block:
# block 0 has an idle-ish DVE (no injected post/FFN work) so it takes more
EXP_DVE = (frozenset({1, 3, 5, 7, 9, 11, 13, 15}),
           frozenset({1, 4, 6, 9, 11, 14}))


def emit(ctx: ExitStack, tc, io):
    nc = tc.nc

    xT, xo = io["xT"], io["xo"]
    wq, wk, wv, wo, w1, w2 = io["wq"], io["wk"], io["wv"], io["wo"], io["w1"], io["w2"]
    out = io["out"]

    const = ctx.enter_context(tc.tile_pool(name="const", bufs=1))
    persist = ctx.enter_context(tc.tile_pool(name="persist", bufs=1))
    exp_pool = ctx.enter_context(tc.tile_pool(name="exp", bufs=4))
    cxu_pool = ctx.enter_context(tc.tile_pool(name="cxu", bufs=6))
    work = ctx.enter_context(tc.tile_pool(name="work", bufs=3))
    stat = ctx.enter_context(tc.tile_pool(name="stat", bufs=4))
    norm = ctx.enter_context(tc.tile_pool(name="norm", bufs=1))
    xo_pool = ctx.enter_context(tc.tile_pool(name="xo", bufs=2))
    out_pool = ctx.enter_context(tc.tile_pool(name="out", bufs=2))

    mm_ps = ctx.enter_context(tc.tile_pool(name="mm_ps", bufs=2, space="PSUM"))
    sc_ps = ctx.enter_context(tc.tile_pool(name="sc_ps", bufs=2, space="PSUM"))
    ctx_ps = ctx.enter_context(tc.tile_pool(name="ctx_ps", bufs=1, space="PSUM"))

    # ---- persistent SBUF arrays ----
    # All weight/activation dram tensors are host-prearranged into the exact
    # SBUF layout (partition-major) so DMAs are long contiguous rows; each
    # array is split into per-k chunks so the transfers spread across many
    # DMA queues (~22 GB/s each). Issue order favors Q-projection inputs so
    # the PE starts early.
    wq_sb = persist.tile([P, KD, D], BF16, tag="wq")
    nc.sync.dma_start(wq_sb[:], wq[:, :, :])
    xT_sb = persist.tile([P, KD, S], BF16, tag="xT")
    for k in range(KD):
        nc.sync.dma_start(xT_sb[:, k, 0:T1], xT[:, k, 0:T1])
    wk_sb = persist.tile([P, KD, D], BF16, tag="wk")
    nc.sync.dma_start(wk_sb[:], wk[:, :, :])
    for k in range(KD):
        nc.sync.dma_start(xT_sb[:, k, T1:S], xT[:, k, T1:S])
    wv_sb = persist.tile([P, KD, D], BF16, tag="wv")
    nc.sync.dma_start(wv_sb[:], wv[:, :, :])
    wo_sb = persist.tile([P, KD, D], BF16, tag="wo")
    nc.sync.dma_start(wo_sb[:], wo[:, :, :])

    kt_sb = persist.tile([P, NPAIR, S], BF16, tag="kt")
    qt_sb = persist.tile([P, NPAIR, T1], BF16, tag="qt")
    # V (fp8) with per-head ones column, t2 tiles paired on the j axis for
    # DoubleRow: [t2 128, t2pair, j, head, 80]; cols 64 are ones, 65-79 pad.
    ve_sb = persist.tile([P, NTP, 2, H, DVP], FP8, tag="ve")
    nc.vector.memset(ve_sb[:, :, :, :, DK:DK + 1], 1.0)
    ctxT_sb = persist.tile([P, NPAIR, T1], BF16, tag="ctxT")
    x1_sb = persist.tile([P, NT1, D], BF16, tag="x1")
    x1T_sb = persist.tile([P, KD, T1], BF16, tag="x1T")
    h1T_sb = persist.tile([P, NDFF, T1], BF16, tag="h1T")

    # ---- constants ----
    # per-partition bias tiles (feature-major evictions) -- first so the Q/K
    # evictions are never gated on the gpsimd DMA queue
    bqt = const.tile([P, KD], F32)
    nc.gpsimd.dma_start(bqt[:], io["bq"][:].rearrange("(m p) -> p m", p=P))
    bkt = const.tile([P, KD], F32)
    nc.gpsimd.dma_start(bkt[:], io["bk"][:].rearrange("(m p) -> p m", p=P))
    b1t = const.tile([P, NDFF], F32)
    nc.gpsimd.dma_start(b1t[:], io["b1"][:].rearrange("(m p) -> p m", p=P))

    ident_sb = const.tile([P, P], BF16)
    make_identity(nc, ident_sb[:])
    # HAM warm-up: the PE would otherwise idle ~10us waiting for the first
    # weight DMAs and start the projections cold (K=4/8, 1.2GHz). ident_sb
    # needs no DMA, so a chain of back-to-back identity matmuls spans the
    # wait and flips the clock gate to 8/8 before real work arrives.
    warm_ps = mm_ps.tile([P, P], F32, tag="mm")
    for _ in range(150):
        nc.tensor.matmul(warm_ps[:], ident_sb[:], ident_sb[:],
                         start=True, stop=True)
    ones1_sb = const.tile([1, P], BF16)
    nc.vector.memset(ones1_sb[:], 1.0)
    b2r_sb = const.tile([1, D], BF16)
    nc.gpsimd.dma_start(b2r_sb[:], io["b2"][:].unsqueeze(0))
    eps_sb = const.tile([P, 1], F32)
    nc.vector.memset(eps_sb[:], EPS)
    # per-pair 1/Z partition-broadcast indicator: kron(eye(2), ones(64))
    ind2_sb = const.tile([2, P], BF16)
    nc.gpsimd.dma_start(ind2_sb[:], io["ind2"][:, :])

    # free-axis broadcast tiles (token-major ops)
    def bc_tile(name):
        t = const.tile([P, D], BF16, tag=f"bc_{name}")
        a = io[name][:]
        bcast = bass.AP(tensor=a.tensor, offset=a.offset, ap=[[0, P]] + list(a.ap))
        nc.gpsimd.dma_start(t[:], bcast)
        return t

    g1b = bc_tile("g1")
    be1b = bc_tile("be1")
    g2b = bc_tile("g2")
    be2b = bc_tile("be2")

    # ---- projections ----
    # Q^T (feature-major, head-pairs stacked): [dk-pair 128, t1];
    # only m=0 (pair 0) upfront, m=1..3 injected into block-0 attention
    def qproj(m, nb):
        ps = mm_ps.tile([P, 512], F32, tag="mm")
        for k in range(KD):
            nc.tensor.matmul(
                ps[:],
                wq_sb[:, k, m * P:(m + 1) * P],
                xT_sb[:, k, nb * 512:(nb + 1) * 512],
                start=(k == 0),
                stop=(k == KD - 1),
            )
        nc.scalar.activation(
            qt_sb[:, m, nb * 512:(nb + 1) * 512], ps[:], AF.Identity,
            bias=bqt[:, m:m + 1],
        )

    for m in range(KD):
        for nb in range(NB1):
            qproj(m, nb)
    # K^T -- m=0 (pair 0) upfront; m=1..3 injected into block-0 attention
    def kproj(m, nb):
        ps = mm_ps.tile([P, 512], F32, tag="mm")
        for k in range(KD):
            nc.tensor.matmul(
                ps[:],
                wk_sb[:, k, m * P:(m + 1) * P],
                xT_sb[:, k, nb * 512:(nb + 1) * 512],
                start=(k == 0),
                stop=(k == KD - 1),
            )
        nc.scalar.activation(
            kt_sb[:, m, nb * 512:(nb + 1) * 512], ps[:], AF.Identity,
            bias=bkt[:, m:m + 1],
        )

    for nb in range(NBS):
        kproj(0, nb)
    # V (token-major fp8, per-head strided [h, dv] groups in j-planes);
    # tiles 0..1 upfront, the rest injected into block-0 pair 0 so the
    # projection matmuls fill the exp-bound attention window.
    def vproj(i):
        ps = mm_ps.tile([P, 512], F32, tag="mm")
        for k in range(KD):
            nc.tensor.matmul(
                ps[:],
                xT_sb[:, k, i * P:(i + 1) * P],
                wv_sb[:, k, :],
                start=(k == 0),
                stop=(k == KD - 1),
            )
        # bv is folded into the residual host-side (softmax weights sum to 1,
        # so ctx = ctx_nobias + bv and bv@Wo+bo joins xo); plain fp8 evict.
        nc.scalar.activation(
            ve_sb[:, i // 2, i % 2, :, 0:DK],
            ps[:].rearrange("p (h d) -> p h d", h=H),
            AF.Identity,
        )

    vproj(0)
    vproj(1)

    # W1 shares the xT slot (xT dead once the injected projections finish);
    # the prefetch DMAs go on the gpsimd queue (their wait on the xT slot
    # must not block sync's small Z/stg/out DMAs) and are emitted at block-0
    # pair 3 via prefetch_w().
    w1_sb = persist.tile([P, KD, DFF], BF16, tag="xT")
    w2_sb = persist.tile([P, NDFF, D], BF16, tag="w2")

    def prefetch_w():
        nc.gpsimd.dma_start(w1_sb[:], w1[:, :, :])
        nc.gpsimd.dma_start(w2_sb[:], w2[:, :, :])

    def layer_norm(r, gb, beb, dest, eng=None):
        """dest = LN(r)*g + be; r is f32 SBUF [128, D]."""
        eng = eng or nc.gpsimd
        st = stat.tile([P, 6], F32, tag="st")
        nc.vector.bn_stats(st[:], r[:])
        mv = stat.tile([P, 2], F32, tag="mv")
        nc.vector.bn_aggr(mv[:], st[:])
        lnv = stat.tile([P, 1], F32, tag="lnv")
        nc.scalar.activation(lnv[:], mv[:, 1:2], AF.Ln, bias=eps_sb[:, 0:1])
        rstd = stat.tile([P, 1], F32, tag="rstd")
        nc.scalar.activation(rstd[:], lnv[:], AF.Exp, scale=-0.5)
        xc = work.tile([P, D], F32, tag="xc")
        nc.vector.tensor_scalar(
            xc[:], r[:], mv[:, 0:1], rstd[:], op0=OP.subtract, op1=OP.mult
        )
        xg = work.tile([P, D], F32, tag="xg")
        eng.tensor_tensor(xg[:], xc[:], gb[:], OP.mult)
        eng.tensor_tensor(dest, xg[:], beb[:], OP.add)

    post_stats = {}
    # prefetch all residual tiles early (DMA queues are idle mid-kernel)
    xo_tiles = {}
    for t1t in range(NT1):
        xo_t = xo_pool.tile([P, D], F32)
        nc.gpsimd.dma_start(xo_t[:], xo[t1t * P:(t1t + 1) * P, :])
        xo_tiles[t1t] = xo_t

    def post_attn1(t1t, eng=None):
        """Wo + residual (staged bf16 in the x1 slot) + bn stats."""
        eng = eng or nc.gpsimd
        ao = mm_ps.tile([P, 512], F32, tag="mm")
        for k in range(NPAIR):
            nc.tensor.matmul(
                ao[:],
                ctxT_sb[:, k, t1t * P:(t1t + 1) * P],
                wo_sb[:, k, :],
                start=(k == 0),
                stop=(k == NPAIR - 1),
            )
        xo_t = xo_tiles[t1t]
        rslot = x1_sb[:, t1t, :]
        nc.vector.tensor_tensor(rslot, ao[:], xo_t[:], OP.add)
        st = stat.tile([P, 6], F32, tag="st")
        nc.vector.bn_stats(st[:], rslot)
        mv = stat.tile([P, 2], F32, tag="mv")
        nc.vector.bn_aggr(mv[:], st[:])
        post_stats[t1t] = mv

    def post2_norm(t1t, eng=None):
        """LN1 normalize+affine (in the x1 slot)."""
        eng = eng or nc.gpsimd
        mv = post_stats.pop(t1t)
        rslot = x1_sb[:, t1t, :]
        lnv = stat.tile([P, 1], F32, tag="lnv")
        nc.scalar.activation(lnv[:], mv[:, 1:2], AF.Ln, bias=eps_sb[:, 0:1])
        rstd = stat.tile([P, 1], F32, tag="rstd")
        nc.scalar.activation(rstd[:], lnv[:], AF.Exp, scale=-0.5)
        xc = work.tile([P, D], F32, tag="xc")
        nc.vector.tensor_scalar(
            xc[:], rslot, mv[:, 0:1], rstd[:], op0=OP.subtract, op1=OP.mult
        )
        xg = work.tile([P, D], F32, tag="xg")
        eng.tensor_tensor(xg[:], xc[:], g1b[:], OP.mult)
        eng.tensor_tensor(rslot, xg[:], be1b[:], OP.add)

    def post2_T(t1t, eng=None):
        """transpose(x1) -> x1T."""
        eng = eng or nc.gpsimd
        for j in range(KD):
            tp = mm_ps.tile([P, P], BF16, tag="mm")
            nc.tensor.transpose(
                tp[:], x1_sb[:, t1t, j * P:(j + 1) * P], ident_sb[:]
            )
            if eng is nc.vector:
                nc.scalar.copy(x1T_sb[:, j, t1t * P:(t1t + 1) * P], tp[:])
            else:
                nc.vector.tensor_copy(x1T_sb[:, j, t1t * P:(t1t + 1) * P], tp[:])

    def post_attn2(t1t, eng=None):
        post2_norm(t1t, eng)
        post2_T(t1t, eng)

    def ffn1_m(t1b, m, width=512, off=0):
        on_act = t1b == NB1 - 1
        lo = t1b * 512 + off
        if True:
            ps = mm_ps.tile([P, 512], F32, tag="mm")
            for k in range(KD):
                nc.tensor.matmul(
                    ps[:, 0:width],
                    w1_sb[:, k, m * P:(m + 1) * P],
                    x1T_sb[:, k, lo:lo + width],
                    start=(k == 0),
                    stop=(k == KD - 1),
                )
            # h1 = relu(ps + b1); ACT in the tail block (ACT idle there),
            # alternating DVE/ACT in block 0 to avoid a 12us DVE burst
            if on_act or m % 2 == 1:
                nc.scalar.activation(
                    h1T_sb[:, m, lo:lo + width], ps[:, 0:width], AF.Relu,
                    bias=b1t[:, m:m + 1],
                )
            else:
                nc.vector.tensor_scalar(
                    h1T_sb[:, m, lo:lo + width], ps[:, 0:width],
                    b1t[:, m:m + 1], 0.0, op0=OP.add, op1=OP.max,
                )

    def ffn1(t1b, width=512, off=0):
        for m in range(NDFF):
            ffn1_m(t1b, m, width, off)

    def ffn2(t1t, eng=None):
        eng = eng or nc.vector
        ff = mm_ps.tile([P, 512], F32, tag="mm")
        for k in range(NDFF):
            nc.tensor.matmul(
                ff[:],
                h1T_sb[:, k, t1t * P:(t1t + 1) * P],
                w2_sb[:, k, :],
                start=(k == 0),
                stop=False,
            )
        # + b2 via a K=1 broadcast matmul (frees the tail engine chains)
        nc.tensor.matmul(
            ff[:], ones1_sb[0:1, :], b2r_sb[0:1, :], start=False, stop=True,
        )
        r = work.tile([P, D], F32, tag=f"r2{t1t % 2}")
        nc.vector.tensor_tensor(r[:], ff[:], x1_sb[:, t1t, :], OP.add)
        o = out_pool.tile([P, D], F32)
        layer_norm(r, g2b, be2b, o[:], eng=eng)
        nc.sync.dma_start(out[t1t * P:(t1t + 1) * P, :], o[:])

    # ---- attention (t1-block outer so downstream work pipelines) ----
    pending_norm = []
    for t1b in range(NB1):
        t1s = slice(t1b * 512, (t1b + 1) * 512)
        # block-0 post/FFN spread across block-1's t2 slots one unit at a
        # time (burst-free engine queues); post_attn1 must come after the
        # block-0 pair-3 normalize flush (pair-0 slot 9).
        inj = {}
        if t1b == 1:
            inj[(0, 10)] = lambda: post_attn1(0)
            inj[(0, 12)] = lambda: post_attn1(1)
            inj[(0, 14)] = lambda: post_attn1(2)
            inj[(1, 0)] = lambda: post_attn1(3)
            inj[(1, 2)] = lambda: post2_norm(0)
            inj[(1, 4)] = lambda: post2_norm(1)
            inj[(1, 6)] = lambda: post2_norm(2)
            inj[(1, 8)] = lambda: post2_norm(3)
            inj[(1, 10)] = lambda: post2_T(0)
            inj[(1, 12)] = lambda: post2_T(1)
            inj[(1, 14)] = lambda: post2_T(2)
            inj[(2, 0)] = lambda: post2_T(3)
            for mm_ in range(1, 16):
                inj[(2, mm_)] = (lambda m=mm_ - 1: ffn1_m(0, m))
            inj[(3, 0)] = lambda: ffn1_m(0, 15)
            inj[(3, 2)] = lambda: ffn2(0, eng=nc.gpsimd)
            inj[(3, 9)] = lambda: ffn2(1, eng=nc.gpsimd)

        # Software pipeline: the DoubleRow ctx matmuls for DR-slot q run two
        # slots after their exps were emitted (at slot q+2), so the PE never
        # stalls on an in-flight exp. cx banks for pair p are freed by the
        # cu evictions emitted right after DR(p, tp=7) -- one slot before
        # DR(p+1, tp=0) needs them.
        cx = {}

        def emit_dr(q):
            p_, tp_ = divmod(q, NTP)
            hA_, hB_ = 2 * p_, 2 * p_ + 1
            if tp_ == 0:
                cxA = ctx_ps.tile([DV1, 512], F32, tag="cxA")
                cxB = ctx_ps.tile([DV1, 512], F32, tag="cxB")
                cx[p_] = (cxA, cxB)
            cxA, cxB = cx[p_]
            eAB = etile.pop(q)
            first, last = tp_ == 0, tp_ == NTP - 1
            nc.tensor.matmul(
                cxA[:, :], ve_sb[:, tp_, :, hA_, 0:DV1], eAB[:, 0, :, :],
                start=first, stop=last, perf_mode=PM.DoubleRow,
            )
            nc.tensor.matmul(
                cxB[:, :], ve_sb[:, tp_, :, hB_, 0:DV1], eAB[:, 1, :, :],
                start=first, stop=last, perf_mode=PM.DoubleRow,
            )
            if last:
                # evict unnormalized ctx (bf16, frees the cx banks), gather Z
                # rows and start 1/Z = exp(-ln Z) on ACT. The PE-side pieces
                # (bch broadcast matmuls + normalize muls) are DEFERRED a few
                # slots so the PE FIFO never blocks on this chain.
                zall2 = norm.tile([2, 512], F32, tag=f"z2_{p_ % 2}")
                cus = {}
                for h, cxt in ((hA_, cxA), (hB_, cxB)):
                    cu = cxu_pool.tile([64, 512], BF16, tag="cu")
                    nc.vector.tensor_copy(cu[:], cxt[0:64, :])
                    zst = norm.tile([P, 512], F32, tag=f"zst{h % 2}")
                    nc.scalar.copy(zst[64:65, :], cxt[64:65, :])
                    nc.sync.dma_start(zall2[h % 2:h % 2 + 1, :], zst[64:65, :])
                    cus[h] = cu
                lz2 = norm.tile([2, 512], F32, tag=f"lz_{p_ % 2}")
                nc.scalar.activation(lz2[:], zall2[:], AF.Ln)
                rz2 = norm.tile([2, 512], BF16, tag=f"rz_{p_ % 2}")
                nc.scalar.activation(rz2[:], lz2[:], AF.Exp, scale=-1.0)
                ts_ = t1s

                def do_norm(p=p_, hA=hA_, hB=hB_, rz=rz2, cus=cus, ts=ts_):
                    for h in (hA, hB):
                        odd = h % 2
                        bch = mm_ps.tile([64, 512], F32, tag="mm")
                        nc.tensor.matmul(
                            bch[:], ind2_sb[:, odd * DK:(odd + 1) * DK],
                            rz[:, :], start=True, stop=True,
                        )
                        if not odd:
                            nc.vector.tensor_tensor(
                                ctxT_sb[0:64, p, ts], cus[h][:], bch[:], OP.mult
                            )
                        else:
                            stg = work.tile([64, 512], BF16, tag="stg")
                            nc.vector.tensor_tensor(stg[:], cus[h][:], bch[:],
                                                    OP.mult)
                            nc.sync.dma_start(ctxT_sb[64:128, p, ts], stg[:])

                pending_norm.append(do_norm)

        etile = {}
        for pair in range(NPAIR):
            for t2 in range(NT2):
                t2s = slice(t2 * P, (t2 + 1) * P)
                tp, j = divmod(t2, 2)
                q = pair * NTP + tp
                sAB = sc_ps.tile([P, 2, 512], F32, tag="s")
                nc.tensor.matmul(
                    sAB[:, 0, :], kt_sb[0:64, pair, t2s], qt_sb[0:64, pair, t1s],
                    start=True, stop=True, tile_position=(0, 0),
                )
                nc.tensor.matmul(
                    sAB[:, 1, :], kt_sb[64:128, pair, t2s], qt_sb[64:128, pair, t1s],
                    start=True, stop=True, tile_position=(64, 0),
                    skip_group_check=True,
                )
                # softmax numerator in fp8, split across ACT (LUT exp) and
                # DVE (Schraudolph bit-trick via round-to-int8)
                if j == 0:
                    eAB = exp_pool.tile([P, 2, 2, 512], FP8, tag="e")
                    etile[q] = eAB
                if t2 % NT2 in EXP_DVE[t1b]:
                    nc.vector.tensor_scalar(
                        etile[q][:, :, j, :].bitcast(I8), sAB[:, :, :], A8, B8,
                        op0=OP.mult, op1=OP.add,
                    )
                else:
                    nc.scalar.activation(
                        etile[q][:, :, j, :], sAB[:, :, :], AF.Exp
                    )
                if j == 1 and q >= 2:
                    emit_dr(q - 2)
                    if (q - 2) % NTP == 2 and pending_norm:
                        pending_norm.pop(0)()
                if t1b == 0:
                    # fill block-0's exp-bound window with deferred projections
                    if pair == 0 and 1 <= t2 <= 7:
                        vproj(2 * t2)
                        vproj(2 * t2 + 1)
                    elif pair < 3 and 9 <= t2 <= 12:
                        kproj(pair + 1, t2 - 9)
                    elif pair == 3 and t2 == 0:
                        prefetch_w()
                fn = inj.get((pair, t2))
                if fn is not None:
                    fn()
        emit_dr(NPAIR * NTP - 2)
        emit_dr(NPAIR * NTP - 1)

    # tail: block-1 post + FFN, interleaved so LN chains overlap matmuls and
    # run on two engines (DVE / GpSimd) concurrently; ffn1 is split in half
    # so ffn2+LN2+store of tiles 4,5 overlap ffn1 of tiles 6,7
    ffn2(2, eng=nc.gpsimd)
    ffn2(3, eng=nc.gpsimd)
    while pending_norm:
        pending_norm.pop(0)()
    post_attn1(4, eng=nc.vector)
    post_attn1(5, eng=nc.vector)
    post2_norm(4, eng=nc.vector)
    post2_norm(5, eng=nc.gpsimd)
    post_attn1(6, eng=nc.vector)
    post_attn1(7, eng=nc.vector)
    post2_T(4, eng=nc.vector)
    post2_T(5, eng=nc.gpsimd)
    post2_norm(6, eng=nc.vector)
    post2_norm(7, eng=nc.gpsimd)
    ffn1(1, width=256, off=0)
    post2_T(6, eng=nc.vector)
    post2_T(7, eng=nc.gpsimd)
    ffn2(4, eng=nc.vector)
    ffn1(1, width=256, off=256)
    ffn2(5, eng=nc.gpsimd)
    ffn2(6, eng=nc.gpsimd)
    ffn2(7, eng=nc.vector)


def _patch_act_tables():
    """Force every ACT op onto the natural_log_exp_and_others table set so
    the kernel pays one ACT_TABLE_LOAD instead of thrashing between the
    per-function default sets (Exp<->Ln cost 33 loads / 42us)."""
    import functools
    import concourse.hw_specs as hw_specs

    if getattr(hw_specs, "_nle_only", False):
        return
    orig = hw_specs.get_activation_tables

    @functools.cache
    def nle_only(arch):
        tabs = orig(arch)
        return {
            k: (v if k == "natural_log_exp_and_others" else set())
            for k, v in tabs.items()
        }

    hw_specs.get_activation_tables = nle_only
    hw_specs._nle_only = True
    # bacc imported the symbol directly
    if getattr(bacc, "get_activation_tables", None) is not None:
        bacc.get_activation_tables = nle_only


def build_program():
    _patch_act_tables()
    nc = bacc.Bacc("TRN2", target_bir_lowering=False, debug=False, num_devices=NCORES)
    io = {}
    io["xT"] = nc.dram_tensor("xT", [P, KD, S], BF16, kind="ExternalInput").ap()
    io["xo"] = nc.dram_tensor("xo", [T1, D], F32, kind="ExternalInput").ap()
    for name, shape in [
        ("wq", [P, KD, D]), ("wk", [P, KD, D]), ("wv", [P, KD, D]),
        ("wo", [P, KD, D]), ("w1", [P, KD, DFF]), ("w2", [P, NDFF, D]),
    ]:
        io[name] = nc.dram_tensor(name, shape, BF16, kind="ExternalInput").ap()
    for name, n in [
        ("bq", D), ("bk", D), ("bv", D), ("bo", D), ("b1", DFF), ("b2", D),
        ("g1", D), ("be1", D), ("g2", D), ("be2", D),
    ]:
        io[name] = nc.dram_tensor(name, [n], F32, kind="ExternalInput").ap()
    io["ind2"] = nc.dram_tensor("ind2", [2, P], BF16, kind="ExternalInput").ap()
    io["out"] = nc.dram_tensor("out", [T1, D], F32, kind="ExternalOutput").ap()

    with tile.TileContext(nc) as tc:
        with ExitStack() as ctx:
            emit(ctx, tc, io)
    nc.compile()
    return nc


def make_in_maps(x, Wq, bq, Wk, bk, Wv, bv, Wo, bo, W1, b1, W2, b2,
                 g1, be1, g2, be2):
    bf = ml_dtypes.bfloat16
    f32 = np.float32
    scale = 1.0 / np.sqrt(DK)

    def lay(w):
        # [D_in, M] -> partition-major SBUF layout [P, D_in//P, M]
        w = np.asarray(w)
        kd = w.shape[0] // P
        return np.ascontiguousarray(w.reshape(kd, P, w.shape[1]).transpose(1, 0, 2))

    shared = {
        "wq": lay((np.asarray(Wq, f32) * scale).astype(bf)),
        "wk": lay(np.asarray(Wk, f32).astype(bf)),
        "wv": lay(np.asarray(Wv, f32).astype(bf)),
        "wo": lay(np.asarray(Wo, f32).astype(bf)),
        "w1": lay(np.asarray(W1, f32).astype(bf)),
        "w2": lay(np.asarray(W2, f32).astype(bf)),
        "bq": (np.asarray(bq, f32) * scale),
        "bk": np.asarray(bk, f32), "bv": np.asarray(bv, f32),
        "bo": np.asarray(bo, f32), "b1": np.asarray(b1, f32),
        "b2": np.asarray(b2, f32), "g1": np.asarray(g1, f32),
        "be1": np.asarray(be1, f32), "g2": np.asarray(g2, f32),
        "be2": np.asarray(be2, f32),
        "ind2": np.kron(np.eye(2, dtype=f32), np.ones((1, DK), f32)).astype(bf),
    }
    x = np.asarray(x, f32)
    # softmax weights sum to 1, so bv rides through attention additively:
    # attn_out = ctx_nobias @ Wo + (bv @ Wo + bo); fold into the residual.
    bo_f = (np.asarray(bo, f32) + np.asarray(bv, f32) @ np.asarray(Wo, f32))
    in_maps = []
    for c in range(NCORES):
        b, half = divmod(c, 2)
        xb = x[b]                                    # [S, D] f32
        sl = slice(half * T1, (half + 1) * T1)
        ot = slice((1 - half) * T1, (2 - half) * T1)
        # own tokens FIRST: attention is order-invariant over context, and
        # this lets the shared program read Q's inputs at a fixed offset
        xperm = np.concatenate([xb[sl], xb[ot]], axis=0)
        m = dict(shared)
        m["xT"] = lay(np.ascontiguousarray(xperm.T).astype(bf))
        m["xo"] = np.ascontiguousarray(xb[sl]) + bo_f  # residual with bo folded
        in_maps.append(m)
    return in_maps


_prog_cache = {}


def get_program():
    if "nc" not in _prog_cache:
        _prog_cache["nc"] = build_program()
    return _prog_cache["nc"]


def kernel(**inputs) -> np.ndarray:
    nc = get_program()
    in_maps = make_in_maps(**inputs)
    res = run_bass_kernel_spmd(nc, in_maps, core_ids=list(range(NCORES)))
    out = np.empty((B, S, D), np.float32)
    for c in range(NCORES):
        b, half = divmod(c, 2)
        out[b, half * T1:(half + 1) * T1] = res.results[c]["out"]
    return out


if __name__ == "__main__":
    rng = np.random.default_rng(0)
    print("building program...")
    get_program()
    print("built")

